# revision 67
# baseline (speedup 1.0000x reference)
"""MultiHeadSelfAttention Trainium2 Bass kernel (v16, ~359us HW).

Shapes (hardcoded): B=8, N=2048, E=512, H=8 heads, D=64 head dim.
Sharding: data-parallel over batch -> one batch item per NeuronCore (8 cores),
no collectives needed.

Design (vs the ~420us v2 baseline):
  - The kernel's floor is the softmax exp: 256 ScalarE ACTIVATEs of
    [128,1024] f32->bf16 at ~1148ns = ~294us, serial on the one ACT engine.
    (DVE tensor_tensor pow fails the TRN2 ISA check; GpSimd pow compiles
    but runs as a ~169us/tile DSP loop -- both measured dead ends.)
    Everything else is arranged to hide under that wall.
  - Inputs are cast to bf16 on the HOST (the kernel computed in bf16 SBUF
    tiles anyway), so all stage-0 transposes run as DMA xbar transposes
    straight from DRAM, one call per tensor (3D out AP scatters the
    transpose rows across partition x e-tile; batching saves ~0.7us of
    fixed descriptor-gen ucode per avoided call). The ucode (~34us total)
    runs serially on the SP DGE queue and is the ramp; it is ordered
    K-half, Q-chunk0, V-chunk0, K-rest, V-rest, Q-rest so the first exp
    fires ~31us in and the first PVs unblock early (the gate is ucode +
    the K transfer + cold-p-state projs). (The ACT DGE is also a legal
    xbar queue but corrupts transposes on hw -- measured NaNs.) Chunk 1 defers its PVs 8 kts behind the exps (p2 ring is 12
    deep) so exps run while V^T lands.
  - Chunk order is qc-major (qc outer, head-pair inner) so output
    projection y-tiles unlock after every 4th chunk and drip 1 unit/chunk
    across 12 chunks instead of piling onto the last head-pair; the last
    q-chunk's 4 y units overlap the flush by running their et0-2 partial
    matmuls under the final norm's DMA latency.
  - Per-chunk softmax normalization is split across the chunk boundary:
    the last two PVs + o2 drains + reciprocal round-trip run at kt0 of
    the NEXT chunk (after its first scores), and the denominator
    broadcast (a bf16 ones-column PE matmul -- compute engines cannot
    partition-broadcast) + multiply run at kt6, after the round-trip
    (~5us latency) has landed, so the in-order PE stream never blocks.
    The reciprocal itself runs on a DMA-gathered [128,8] tile: DVE
    reciprocal costs ~6.5ns per per-partition element, so wide or
    single-partition recips are 3.4us while this one is ~60ns.
  - Output projection runs in direct (non-transposed) orientation:
    y[n-tile] = sum_et oTn[et][:, n-block]^T @ WoT[et] + bo. No
    de-transpose, no cast: TT bias-add writes f32, DMA stores it.
  - Scores arrive pre-scaled by log2e/tau (folded into Wq on the host);
    the exp ACTIVATE uses scale=LN2. Attention core: scores 2 heads/kt
    via PE row tiling into one [128,1024] PSUM tile -> one exp per kt;
    v_aug ones-column gives softmax denominators for free.
"""

import sys

for _p in ("/opt/trn_rl_repo",):
    if _p not in sys.path:
        sys.path.insert(0, _p)

import numpy as np
from collections import deque
from contextlib import ExitStack

import concourse.bass as bass
import concourse.bacc as bacc
import concourse.mybir as mybir
import concourse.tile as tile

B, N, E = 8, 2048, 512
H, D = 8, 64
P = 128          # partitions
ET = E // P      # 4 e-tiles
NT = N // P      # 16 n-tiles
QC = 512         # q chunk in attention
NQC = N // QC    # 4
HV = 65          # head dim + ones column
FP32 = mybir.dt.float32
BF16 = mybir.dt.bfloat16
NCORES = 8

AF = mybir.ActivationFunctionType
ALU = mybir.AluOpType
LOG2E = 1.4426950408889634
LN2 = 0.6931471805599453


def _build() -> bass.Bass:
    nc = bacc.Bacc(trn_type="TRN2")

    dQ = nc.dram_tensor("Q", [N, E], BF16, kind="ExternalInput")
    dK = nc.dram_tensor("K", [N, E], BF16, kind="ExternalInput")
    dV = nc.dram_tensor("V", [N, E], BF16, kind="ExternalInput")
    dW = {
        "q": nc.dram_tensor("Wq", [E, E], BF16, kind="ExternalInput"),
        "k": nc.dram_tensor("Wk", [E, E], BF16, kind="ExternalInput"),
        "v": nc.dram_tensor("Wv", [E, E], BF16, kind="ExternalInput"),
        "o": nc.dram_tensor("Wo", [E, E], BF16, kind="ExternalInput"),
    }
    dbo = nc.dram_tensor("bo", [E], FP32, kind="ExternalInput")
    dout = nc.dram_tensor("out", [N, E], FP32, kind="ExternalOutput")

    with tile.TileContext(nc) as tc, ExitStack() as ctx:
        _body(ctx, tc, dQ, dK, dV, dW, dbo, dout)
    nc.finalize()
    return nc


def _body(ctx, tc, dQ, dK, dV, dW, dbo, dout):
    nc = tc.nc
    sdma = nc.sync.dma_start    # SP DGE queue (ACT's DGE corrupts xbar transposes)

    const = ctx.enter_context(tc.tile_pool(name="const", bufs=1))
    # 12 x [128, N] bf16 slots reused across phases:
    #   stage 0: Q^T (big_0..3) / K^T (big_4..7) / V^T (big_8..11)
    #   attn: oTn (big_0..3)
    big = ctx.enter_context(tc.tile_pool(name="big", bufs=1))
    proj = ctx.enter_context(tc.tile_pool(name="proj", bufs=1))
    # PSUM budget (8 banks of [128,512] f32):
    #   s2 ([128,1024] f32, bufs=3) -> 6 banks: attention scores; proj,
    #       out-proj and norm-broadcast tiles ride the same ring
    #   o2e/o2o ([65,512] f32 PV accum, bufs=1) -> 2 banks
    psum = ctx.enter_context(tc.tile_pool(name="psum", bufs=1, space="PSUM"))
    stage = ctx.enter_context(tc.tile_pool(name="stage", bufs=4))
    # deep p2 ring: exp may run ~12 kts ahead of PV, so the first chunk's
    # exps keep running at scores-pace across the V^T arrival gate
    p2pool = ctx.enter_context(tc.tile_pool(name="p2pool", bufs=12))
    rnpool = ctx.enter_context(tc.tile_pool(name="rnpool", bufs=2))

    # bias replicated across all partitions (for the direct-orientation
    # output projection the bias varies along the free dim); dispatched
    # after the stage-0 transposes (first needed by y units, chunk 5+)
    bo_full = const.tile([P, E], FP32, name="bo_full", tag="bo_full")

    # all-ones [128, 64] so a [1, 64] slice exists at any base partition
    # (matmul requires lhsT and rhs to share their base partition)
    ones_bf = const.tile([P, 64], BF16, name="ones_bf", tag="ones_bf")
    nc.gpsimd.memset(ones_bf, 1.0)

    # ---- stage 0: all transposes via DMA xbar, straight from DRAM ----
    # One call per tensor: the 2D transpose's rows scatter across
    # (partition, e-tile) via a 3D out AP, saving ~0.7us of serial DGE
    # ucode fixed cost per avoided call (20 calls' worth).
    wt = {}
    for wname in ("q", "k", "v", "o"):
        wta = const.tile([P, ET * E], BF16, name=f"w{wname}T",
                         tag=f"w{wname}T")
        wt[wname] = [wta[:, c * E:(c + 1) * E] for c in range(ET)]

    def emit_wT(wname):
        # wt[c][i, o] = W[o, c*128+i]
        sdma(out=wt[wname][0].tensor.ap().rearrange(
                 "p (c o) -> p c o", o=E),
             in_=dW[wname][:, :], transpose=True)

    xT = {}
    xTa = {}
    for xname in ("K", "Q", "V"):
        xTa[xname] = big.tile([P, ET * N], BF16, name=f"{xname}Ta",
                              tag=f"{xname}Ta")
        xT[xname] = [xTa[xname][:, et * N:(et + 1) * N] for et in range(ET)]

    def emit_xT(xname, dX, n0=0, n1=N):
        # xT[et][:, n0:n1] = X[n0:n1, et*128:(et+1)*128]^T for all et
        out3 = xTa[xname].rearrange("p (et n) -> p et n", n=N)
        sdma(out=out3[:, :, n0:n1], in_=dX[n0:n1, :], transpose=True)

    # ---- projections ----
    qT = [proj.tile([P, N], BF16, name=f"qT_{m}", tag=f"qT_{m}")
          for m in range(ET)]
    kT = [proj.tile([P, N], BF16, name=f"kT_{m}", tag=f"kT_{m}")
          for m in range(ET)]
    v_aug = [proj.tile([P, H * HV], BF16, name=f"vaug_{nt}",
                       tag=f"vaug_{nt}") for nt in range(NT)]

    def emit_qk_proj(m, c, names=("q", "k")):
        for pname, outs in (("q", qT), ("k", kT)):
            if pname not in names:
                continue
            xtiles = xT[pname.upper()]
            ps = psum.tile([P, 512], FP32, name="pp", tag="s2", bufs=3)
            for et in range(ET):
                nc.tensor.matmul(
                    ps,
                    lhsT=wt[pname][et][:, m * P:(m + 1) * P],
                    rhs=xtiles[et][:, c * 512:(c + 1) * 512],
                    start=(et == 0), stop=(et == ET - 1))
            nc.vector.tensor_copy(outs[m][:, c * 512:(c + 1) * 512], ps)

    def emit_v_proj(nt):
        ps = psum.tile([P, 512], FP32, name="pp", tag="s2", bufs=3)
        for et in range(ET):
            nc.tensor.matmul(
                ps,
                lhsT=xT["V"][et][:, nt * P:(nt + 1) * P],
                rhs=wt["v"][et],
                start=(et == 0), stop=(et == ET - 1))
        va = v_aug[nt].rearrange("p (h c) -> p h c", c=HV)
        nc.vector.tensor_copy(
            va[:, :, 0:D], ps.rearrange("p (h d) -> p h d", d=D))
        nc.gpsimd.memset(va[:, :, D:HV], 1.0)

    # Transpose dispatch order IS the ramp: all xbar descriptor-gen ucode
    # runs serially on the SP DGE queue (~1.3us per [512,128], ~2.5us per
    # [2048,128]; the ACT DGE corrupts transposes on hw -- measured NaNs).
    # First-exp deps come first (wk, K, wq, Q chunk-0 stripes), then the V
    # path; the deep p2 ring keeps exp running while PV waits for v_aug.
    emit_wT("k")
    emit_xT("K", dK, 0, N // 2)  # K^T for kts 0-7: unblocks the first exp
    emit_wT("q")
    emit_xT("Q", dQ, 0, QC)      # q-chunk 0 columns only
    emit_wT("v")
    emit_xT("V", dV, 0, QC)      # V^T for v_aug 0-3: early PV start
    emit_xT("K", dK, N // 2, N)  # rest of K^T (kt8 deadline: pop-paced exps)
    emit_xT("V", dV, QC, N)      # rest of V^T (v_aug 4+ pop at kt7+)
    emit_xT("Q", dQ, QC, N)      # rest of Q^T
    emit_wT("o")
    sdma(out=bo_full, in_=bass.AP(tensor=dbo, offset=0, ap=[[0, P], [1, E]]))
    emit_qk_proj(0, 0, names=("k",))
    emit_qk_proj(0, 1, names=("k",))
    emit_qk_proj(0, 0, names=("q",))  # first exp gate: before kT0 c2/c3
    emit_qk_proj(0, 2, names=("k",))
    emit_qk_proj(0, 3, names=("k",))

    # Deferred stage-0 work, drip-fed under the exp wall as ~1.1us units.
    # f_hp[h]: kT[h] (all chunks) + qT[h] c0 -- needed before head-pair h's
    # first chunk. f_qc[c]: qT[0..3] chunk c -- needed before q-chunk c.
    # chunk1 also carries v_aug 4..15 (its own PVs consume them in kt order).
    f_hp = {h: deque() for h in (1, 2, 3)}
    for h in (1, 2, 3):
        for c in range(ET):
            f_hp[h].append(lambda h=h, c=c: emit_qk_proj(h, c, names=("k",)))
        f_hp[h].append(lambda h=h: emit_qk_proj(h, 0, names=("q",)))
    f_qc = {c: deque() for c in (1, 2, 3)}
    for c in (1, 2, 3):
        for m in range(ET):
            f_qc[c].append(lambda m=m, c=c: emit_qk_proj(m, c, names=("q",)))
    # Chunk 1 carries all of hp1's deps plus every v_aug unit. Its pop
    # schedule keeps V^T-dependent units late (V^T lands ~15us after the
    # first exp; K/Q-dependent units pop first) and its PVs are deferred 8
    # kts behind the exps (the deep p2 ring absorbs the lag), so the
    # in-order PE stream never blocks on the V path while exps run.
    chunk1 = deque()
    for u in list(f_hp[1]):
        chunk1.append(u)
    f_hp[1].clear()
    for nt in range(NT):
        chunk1.append(lambda nt=nt: emit_v_proj(nt))
    # pops per kt for chunk 1: k1c0-3,q1c0 singly, then v0..v15 two per kt
    CHUNK1_POPS = [1, 1, 1, 1, 1, 2, 2, 2, 2, 2, 2, 2, 2, 0, 0, 0]

    # ---- attention ----
    # own slots: with qc-outer order Q^T/K^T stay live (qT/kT c1-3 fillers)
    # long after oTn writes begin, so the stage-0 slots can't be reused
    oTn = [big.tile([P, N], BF16, name=f"oTn_{m}", tag=f"oTn_{m}")
           for m in range(ET)]
    y_fill = deque()  # out-projection units, 1 per chunk once unlocked

    def y_unit(nt):
        ps = psum.tile([P, 512], FP32, name="ps_o", tag="s2", bufs=3)
        for et in range(ET):
            nc.tensor.matmul(
                ps,
                lhsT=oTn[et][:, nt * P:(nt + 1) * P],
                rhs=wt["o"][et],
                start=(et == 0), stop=(et == ET - 1))
        y_sb = stage.tile([P, E], FP32, name="y_sb", tag="y_sb", bufs=2)
        nc.vector.tensor_tensor(y_sb, ps, bo_full, ALU.add)
        sdma(out=dout[nt * P:(nt + 1) * P, :], in_=y_sb)

    def attn_chunk(hp, qc, pend):
        """Both heads of pair hp, q-chunk qc. Row-tiled scores (head-even on
        PE rows 0-63, head-odd on 64-127) stream into one [128,1024] PSUM
        tile -> one [128,1024] exp per kt on ScalarE.

        PV normally lags exp by one kt; the last two PVs + norm prep
        (pend[0]) run at kt0 of the NEXT chunk, after its first scores, so
        the exp stream never waits at a chunk boundary. The norm broadcast
        + multiply (pend[1]) runs at kt6 of the next chunk, after the
        reciprocal's DMA round-trip (~5us latency) has landed. Chunk 1
        defers PVs by 8 kts (the p2 ring is 8 deep) so its exps run while
        V^T / v_aug are still being produced. Returns this chunk's pend."""
        o2 = {}  # allocated lazily at first PV, AFTER prev chunk's tail
        rq = {0: qT[hp][0:64, qc * QC:(qc + 1) * QC],
              1: qT[hp][64:128, qc * QC:(qc + 1) * QC]}
        va = [v_aug[kt].rearrange("p (h c) -> p h c", c=HV)
              for kt in range(NT)]
        pv_q = deque()  # (p2, kt) pending PVs

        def emit_pv(p2, kt):
            if not o2:
                o2[0] = psum.tile([HV, QC], FP32, name="o2e",
                                  tag="o2e", bufs=1)
                o2[1] = psum.tile([HV, QC], FP32, name="o2o",
                                  tag="o2o", bufs=1)
            for h2 in (0, 1):
                nc.tensor.matmul(
                    o2[h2], lhsT=va[kt][:, 2 * hp + h2, :],
                    rhs=p2[:, h2 * QC:(h2 + 1) * QC],
                    start=(kt == 0), stop=(kt == NT - 1),
                    skip_group_check=True)

        first = hp == 0 and qc == 0
        pv_from = 8 if first else 1   # iteration at which PVs start
        for kt in range(NT):
            s2 = psum.tile([P, 2 * QC], FP32, name="s2", tag="s2", bufs=3)
            for h2 in (0, 1):
                nc.tensor.matmul(
                    s2[:, h2 * QC:(h2 + 1) * QC],
                    lhsT=kT[hp][h2 * 64:h2 * 64 + 64, kt * P:(kt + 1) * P],
                    rhs=rq[h2], start=True, stop=True)
            if kt == 0 and pend:
                pend[0]()  # prev chunk's last PVs + norm prep
            if kt >= pv_from and kt <= 14 and pv_q \
                    and len(pv_q) + kt >= NT + 1:
                emit_pv(*pv_q.popleft())  # catch-up path (chunk 1)
            if kt >= pv_from and kt <= 14 and pv_q:
                emit_pv(*pv_q.popleft())
            if kt == 6 and pend:
                pend[1]()  # prev chunk's broadcast + multiply
            p2 = p2pool.tile([P, 2 * QC], BF16, name="p2", tag="p2")
            nc.scalar.activation(p2, s2, AF.Exp, scale=LN2)
            pv_q.append((p2, kt))
            # drip-feed schedule (see filler comments above)
            if first:
                for _ in range(CHUNK1_POPS[kt]):
                    if chunk1:
                        chunk1.popleft()()
            elif qc == 0:
                if kt in (1, 4, 7, 10, 13) and hp < 3 and f_hp[hp + 1]:
                    f_hp[hp + 1].popleft()()
                if kt in (5, 13) and f_qc[1]:
                    f_qc[1].popleft()()
            else:
                if kt == 9 and y_fill:
                    y_fill.popleft()()
                if kt in (5, 13) and qc < 3 and f_qc[qc + 1]:
                    f_qc[qc + 1].popleft()()
        while len(pv_q) > 2:
            emit_pv(*pv_q.popleft())

        # Tail closure (runs at kt0 of the next chunk, after its first
        # scores): last two PVs, o2 drains, and the reciprocal round-trip.
        # DVE reciprocal costs ~6.5ns per per-partition element, so the
        # denominator rows are DMA-gathered to [128,8] (recip = 8 elems/
        # partition, ~60ns), recip'd with bf16 output, and scattered back
        # to a [1,1024] row for the PE ones-matmul broadcast.
        d65 = [stage.tile([HV, QC], FP32, name=f"d65_{h2}",
                          tag=f"d65_{h2}", bufs=2) for h2 in range(2)]

        def tail():
            while pv_q:
                emit_pv(*pv_q.popleft())
            lg = rnpool.tile([P, 8], FP32, name="lg", tag="lg", bufs=2)
            lgb = rnpool.tile([P, 8], BF16, name="lgb", tag="lgb", bufs=2)
            for h2 in range(2):
                nc.vector.tensor_copy(d65[h2], o2[h2])
                sdma(out=lg[:, 4 * h2:4 * h2 + 4], in_=d65[h2][D:HV, :])
            with nc.allow_low_precision(reason="bf16 softmax denominators"):
                nc.vector.reciprocal(lgb, lg)
            for h2 in range(2):
                sdma(out=r_row[0:1, h2 * QC:(h2 + 1) * QC],
                     in_=lgb[:, 4 * h2:4 * h2 + 4])

        r_row = rnpool.tile([1, 2 * QC], BF16, name="r_row", tag="r_row",
                            bufs=2)

        def norm():
            # reciprocal row broadcast across partitions via bf16 ones-mm
            rb_ps = psum.tile([P, QC], FP32, name="rb_ps", tag="s2", bufs=3)
            for h2 in range(2):
                nc.tensor.matmul(
                    rb_ps[h2 * 64:(h2 + 1) * 64, :],
                    lhsT=ones_bf[0:1, :],
                    rhs=r_row[0:1, h2 * QC:(h2 + 1) * QC],
                    start=True, stop=True, skip_group_check=True)
            for h2 in range(2):
                nc.vector.tensor_tensor(
                    oTn[hp][h2 * 64:(h2 + 1) * 64, qc * QC:(qc + 1) * QC],
                    d65[h2][0:D, :], rb_ps[h2 * 64:(h2 + 1) * 64, :],
                    ALU.mult)

        return (tail, norm)

    pend = None
    for qc in range(NQC):
        for hp in range(ET):
            pend = attn_chunk(hp, qc, pend)
            if qc == 0 and hp < 3:
                while f_hp[hp + 1]:  # safety: hp+1's kT/qT must exist
                    f_hp[hp + 1].popleft()()
        while f_qc.get(qc + 1):  # safety: next q-chunk's qT slices
            f_qc[qc + 1].popleft()()
        for nt in range(4 * qc, 4 * qc + 4):
            y_fill.append(lambda nt=nt: y_unit(nt))
    # Flush. The last chunk's 4 y units overlap the norm round-trip: their
    # oTn[0..2] partials (et 0-2) run on the PE while the reciprocal DMAs
    # are in flight -- two partials on the drained o2 banks, two on the s2
    # ring (the ring's 3rd slot stays free for the norm broadcast) -- and
    # only the et3 step waits for oTn[3].
    pend[0]()
    parts = []
    for j in range(4):
        nt = 12 + j
        tag = ("o2e", "o2o", "s2", "s2")[j]
        ps = psum.tile([P, 512], FP32, name="ps_o", tag=tag,
                       bufs=1 if j < 2 else 3)
        for et in range(3):
            nc.tensor.matmul(
                ps, lhsT=oTn[et][:, nt * P:(nt + 1) * P], rhs=wt["o"][et],
                start=(et == 0), stop=False)
        parts.append((nt, ps))
    pend[1]()
    y_fill.clear()
    for nt, ps in parts:
        nc.tensor.matmul(
            ps, lhsT=oTn[3][:, nt * P:(nt + 1) * P], rhs=wt["o"][3],
            start=False, stop=True)
        y_sb = stage.tile([P, E], FP32, name="y_sb", tag="y_sb", bufs=2)
        nc.vector.tensor_tensor(y_sb, ps, bo_full, ALU.add)
        sdma(out=dout[nt * P:(nt + 1) * P, :], in_=y_sb)


_CACHE = {}


def _get_nc() -> bass.Bass:
    if "nc" not in _CACHE:
        _CACHE["nc"] = _build()
    return _CACHE["nc"]


def _prep_core_inputs(inputs: dict) -> list:
    """Host-side prep: bf16 casts + tau/log2e folding into Wq. Returns the
    per-core input maps."""
    import ml_dtypes

    bf16 = ml_dtypes.bfloat16
    tau = float(np.asarray(inputs["tau"]))
    Q = np.asarray(inputs["Q"], dtype=np.float32)
    K = np.asarray(inputs["K"], dtype=np.float32)
    V = np.asarray(inputs["V"], dtype=np.float32)
    Wq = (np.asarray(inputs["Wq"], dtype=np.float32) * (LOG2E / tau)).astype(bf16)
    Wk = np.asarray(inputs["Wk"], dtype=np.float32).astype(bf16)
    Wv = np.asarray(inputs["Wv"], dtype=np.float32).astype(bf16)
    Wo = np.asarray(inputs["Wo"], dtype=np.float32).astype(bf16)
    bo = np.ascontiguousarray(np.asarray(inputs["bo"], dtype=np.float32))
    Qb = Q.astype(bf16)
    Kb = K.astype(bf16)
    Vb = V.astype(bf16)
    in_maps = []
    for b in range(NCORES):
        in_maps.append({
            "Q": np.ascontiguousarray(Qb[b]),
            "K": np.ascontiguousarray(Kb[b]),
            "V": np.ascontiguousarray(Vb[b]),
            "Wq": Wq, "Wk": Wk, "Wv": Wv, "Wo": Wo, "bo": bo,
        })
    return in_maps


def _run(inputs: dict, trace: bool = False):
    """Returns (output [B,N,E] fp32, BassKernelResults)."""
    from concourse.bass_utils import run_bass_kernel_spmd

    mask = inputs.get("attn_mask")
    if mask is not None and not np.all(np.asarray(mask) != 0):
        # Fallback (never hit for the spec'd all-ones mask): host math.
        return _host_reference(
            np.asarray(inputs["Q"], dtype=np.float32),
            np.asarray(inputs["K"], dtype=np.float32),
            np.asarray(inputs["V"], dtype=np.float32),
            np.asarray(mask),
            np.asarray(inputs["Wq"], dtype=np.float32),
            np.asarray(inputs["Wk"], dtype=np.float32),
            np.asarray(inputs["Wv"], dtype=np.float32),
            np.asarray(inputs["Wo"], dtype=np.float32),
            np.asarray(inputs["bo"], dtype=np.float32),
            float(np.asarray(inputs["tau"]))), None

    nc = _get_nc()
    in_maps = _prep_core_inputs(inputs)
    res = run_bass_kernel_spmd(nc, in_maps, list(range(NCORES)), trace=trace)
    out = np.stack([np.asarray(res.results[b]["out"]) for b in range(NCORES)])
    return out.astype(np.float32), res


def _host_reference(Q, K, V, mask, Wq, Wk, Wv, Wo, bo, tau):
    b, n, _ = Q.shape
    q = (Q @ Wq.T).reshape(b, n, H, D).transpose(0, 2, 1, 3)
    k = (K @ Wk.T).reshape(b, n, H, D).transpose(0, 2, 1, 3)
    v = (V @ Wv.T).reshape(b, n, H, D).transpose(0, 2, 1, 3)
    s = np.einsum("bhnd,bhmd->bhnm", q, k) / tau
    s = np.where(mask == 0, -np.inf, s)
    s = s - s.max(axis=-1, keepdims=True)
    e = np.exp(s)
    a = e / e.sum(axis=-1, keepdims=True)
    o = np.einsum("bhnm,bhmd->bhnd", a, v)
    o = o.transpose(0, 2, 1, 3).reshape(b, n, H * D)
    return (o @ Wo.T + bo).astype(np.float32)


def kernel(**inputs) -> np.ndarray:
    out, _ = _run(inputs, trace=False)
    return out
